# revision 1
# baseline (speedup 1.0000x reference)
"""Pair-packed variant: lanes sorted by src; same-row edge pairs share one
src gather ([P,D] fetch serves two compute slots). Phase A = pairs,
phase C = singles. Host unpermutes outputs via orig-id map."""

import numpy as np

import concourse.bass as bass
import concourse.mybir as mybir
import concourse.tile as tile
from concourse import bacc
from concourse.bass import IndirectOffsetOnAxis
from concourse.bass_utils import run_bass_kernel_spmd
from concourse.masks import make_identity
from contextlib import ExitStack

N, D, H = 100000, 128, 128
E_TOTAL = 2000000
NCORES = 8
P = 128
BLK_SLOTS = 4
REG_SUP = 16  # superslots per phase-A region (32 slots)
REG_C = 32  # slots per phase-C region
E_CORE = E_TOTAL // NCORES
S_LANE = -(-E_CORE // P)  # 1954 edges per lane (padded)

F32 = mybir.dt.float32
I32 = mybir.dt.int32
RELU = mybir.ActivationFunctionType.Relu
IDENT = mybir.ActivationFunctionType.Identity


def _block(nc, work, psum_t, psum_h, psum_o, ident, w1_sb, b1_sb, w2_sb, b2_sb,
           ef_srcs, o_stage, e0):
    """Shared 4-slot (512-edge) compute block. ef_srcs: 4 (zs_ap, zd_ap) pairs."""
    EB = BLK_SLOTS * P
    ef = work.tile([P, EB], F32, tag="ef")
    for c, (a, b) in enumerate(ef_srcs):
        nc.vector.tensor_mul(out=ef[:, c * P : (c + 1) * P], in0=a, in1=b)
    efT_ps = psum_t.tile([P, EB], F32)
    for c in range(BLK_SLOTS):
        nc.tensor.transpose(
            out=efT_ps[:, c * P : (c + 1) * P],
            in_=ef[:, c * P : (c + 1) * P],
            identity=ident[:],
        )
    efT = work.tile([P, EB], F32, tag="efT")
    nc.vector.tensor_copy(out=efT[:], in_=efT_ps[:])
    h_ps = psum_h.tile([P, EB], F32)
    nc.tensor.matmul(out=h_ps[:], lhsT=w1_sb[:], rhs=efT[:], start=True, stop=True)
    h_sb = work.tile([P, EB], F32, tag="h")
    nc.scalar.activation(out=h_sb[:], in_=h_ps[:], func=RELU, bias=b1_sb[:, :1],
                         scale=1.0)
    o_ps = psum_o.tile([1, EB], F32)
    nc.tensor.matmul(out=o_ps[:], lhsT=w2_sb[:], rhs=h_sb[:], start=True, stop=True)
    nc.scalar.activation(
        out=o_stage[:1, e0 : e0 + EB], in_=o_ps[:], func=IDENT,
        bias=b2_sb[:1, :1], scale=1.0,
    )


def build_program(nA, nC, n=N):
    nc = bacc.Bacc("TRN2", target_bir_lowering=False, debug=False,
                   enable_asserts=False, num_devices=NCORES)
    X = 3 * nA + 2 * nC
    z_d = nc.dram_tensor("z", [n, D], F32, kind="ExternalInput").ap()
    idx_d = nc.dram_tensor("idx", [P, X], I32, kind="ExternalInput").ap()
    w1_d = nc.dram_tensor("w1", [D, H], F32, kind="ExternalInput").ap()
    b1_d = nc.dram_tensor("b1", [H], F32, kind="ExternalInput").ap()
    w2_d = nc.dram_tensor("w2", [H, 1], F32, kind="ExternalInput").ap()
    b2_d = nc.dram_tensor("b2", [1], F32, kind="ExternalInput").ap()
    out_d = nc.dram_tensor("out", [(2 * nA + nC) * P], F32,
                           kind="ExternalOutput").ap()

    with tile.TileContext(nc) as tc, ExitStack() as ctx:
        const = ctx.enter_context(tc.tile_pool(name="const", bufs=1))
        zpool = ctx.enter_context(tc.tile_pool(name="gather", bufs=3))
        work = ctx.enter_context(tc.tile_pool(name="work", bufs=3))
        stage_pool = ctx.enter_context(tc.tile_pool(name="stage", bufs=2))
        psum_t = ctx.enter_context(tc.tile_pool(name="ps_t", bufs=2, space="PSUM"))
        psum_h = ctx.enter_context(tc.tile_pool(name="ps_h", bufs=2, space="PSUM"))
        psum_o = ctx.enter_context(tc.tile_pool(name="ps_o", bufs=2, space="PSUM"))

        idx_sb = const.tile([P, X], I32)
        nc.sync.dma_start(out=idx_sb[:], in_=idx_d[:, :])
        w1_sb = const.tile([P, H], F32)
        nc.sync.dma_start(out=w1_sb[:], in_=w1_d[:, :])
        b1_sb = const.tile([P, 1], F32)
        nc.sync.dma_start(out=b1_sb[:], in_=b1_d[:, None])
        w2_sb = const.tile([P, 1], F32)
        nc.sync.dma_start(out=w2_sb[:], in_=w2_d[:, :])
        b2_sb = const.tile([1, 1], F32)
        nc.sync.dma_start(out=b2_sb[:1], in_=b2_d[:, None])
        ident = const.tile([P, P], F32)
        make_identity(nc, ident[:])

        def gather(dst_ap, col0):
            nc.gpsimd.indirect_dma_start(
                out=dst_ap, out_offset=None, in_=z_d[:, :],
                in_offset=IndirectOffsetOnAxis(ap=idx_sb[:, col0 : col0 + 1],
                                               axis=0),
            )

        blk = (nc, work, psum_t, psum_h, psum_o, ident, w1_sb, b1_sb, w2_sb, b2_sb)

        # ---- phase A: paired slots (one src gather serves two slots) ----
        for g in range(-(-nA // REG_SUP)):
            t0 = g * REG_SUP
            gsup = min(REG_SUP, nA - t0)
            zs_t = zpool.tile([P, REG_SUP * D], F32, tag="zs")
            zd_t = zpool.tile([P, 2 * REG_SUP * D], F32, tag="zd")
            for t in range(gsup):
                gather(zs_t[:, t * D : (t + 1) * D], t0 + t)
                gather(zd_t[:, (2 * t) * D : (2 * t + 1) * D], nA + t0 + t)
                gather(zd_t[:, (2 * t + 1) * D : (2 * t + 2) * D],
                       2 * nA + t0 + t)
            o_stage = stage_pool.tile([1, 2 * REG_SUP * P], F32, tag="ostage")
            for b in range(gsup * 2 // BLK_SLOTS):
                srcs = []
                for c in range(BLK_SLOTS):
                    s = b * BLK_SLOTS + c
                    w = s // 2
                    srcs.append((zs_t[:, w * D : (w + 1) * D],
                                 zd_t[:, s * D : (s + 1) * D]))
                _block(*blk, srcs, o_stage, b * BLK_SLOTS * P)
            nc.sync.dma_start(
                out=out_d[(2 * t0) * P : (2 * t0 + 2 * gsup) * P][None, :],
                in_=o_stage[:1, : 2 * gsup * P],
            )

        # ---- phase C: single slots ----
        cbase = 3 * nA
        obase = 2 * nA
        for g in range(-(-nC // REG_C)):
            s0 = g * REG_C
            gslots = min(REG_C, nC - s0)
            zc_t = zpool.tile([P, 2 * REG_SUP * D], F32, tag="zd")
            zdc_t = zpool.tile([P, REG_SUP * D] if False else [P, 2 * REG_SUP * D],
                               F32, tag="zs2")
            for k in range(gslots):
                gather(zc_t[:, k * D : (k + 1) * D], cbase + s0 + k)
                gather(zdc_t[:, k * D : (k + 1) * D], cbase + nC + s0 + k)
            o_stage = stage_pool.tile([1, 2 * REG_SUP * P], F32, tag="ostage")
            for b in range(gslots // BLK_SLOTS):
                srcs = []
                for c in range(BLK_SLOTS):
                    s = b * BLK_SLOTS + c
                    srcs.append((zc_t[:, s * D : (s + 1) * D],
                                 zdc_t[:, s * D : (s + 1) * D]))
                _block(*blk, srcs, o_stage, b * BLK_SLOTS * P)
            nc.sync.dma_start(
                out=out_d[(obase + s0) * P : (obase + s0 + gslots) * P][None, :],
                in_=o_stage[:1, : gslots * P],
            )

    nc.compile()
    return nc


def _ragged_pack(vals, mask, width, fill=0):
    Pn, S = mask.shape
    out = np.full((Pn, width), fill, vals.dtype)
    cnt = mask.cumsum(1) - 1
    rows = np.broadcast_to(np.arange(Pn)[:, None], mask.shape)
    out[rows[mask], cnt[mask]] = vals[mask]
    return out


def pack_all(edge_label_index, e_core=E_CORE, s_lane=S_LANE):
    """Per-core (idx [P,3nA+2nC] int32, ORIG [P,2nA+nC] int64) + global nA,nC."""
    src_f = np.asarray(edge_label_index[0], dtype=np.int32)
    dst_f = np.asarray(edge_label_index[1], dtype=np.int32)
    ncores = len(src_f) // e_core
    cores = []
    for c in range(ncores):
        sl = slice(c * e_core, (c + 1) * e_core)
        s = np.zeros(s_lane * P, np.int32)
        t = np.zeros(s_lane * P, np.int32)
        s[:e_core] = src_f[sl]
        t[:e_core] = dst_f[sl]
        order = np.argsort(s, kind="stable").astype(np.int64)
        V = s[order].reshape(P, s_lane)
        Vd = t[order].reshape(P, s_lane)
        EO = order.reshape(P, s_lane)
        eq = V[:, 1:] == V[:, :-1]
        eqx = np.concatenate([eq, np.zeros((P, 1), bool)], 1)
        start = np.concatenate([np.ones((P, 1), bool), ~eq], 1)
        j = np.broadcast_to(np.arange(s_lane)[None, :], (P, s_lane))
        runstart = np.maximum.accumulate(np.where(start, j, 0), 1)
        pos = j - runstart
        pf = eqx & (pos % 2 == 0)
        ps = np.concatenate([np.zeros((P, 1), bool), pf[:, :-1]], 1)
        sg = ~pf & ~ps
        cores.append((V, Vd, EO, pf, ps, sg))
    nA = max(int(x[3].sum(1).max()) for x in cores)
    nC = max(int(x[5].sum(1).max()) for x in cores)
    nA = -(-nA // 2) * 2
    nC = -(-nC // BLK_SLOTS) * BLK_SLOTS
    packed = []
    for V, Vd, EO, pf, ps, sg in cores:
        idx = np.ascontiguousarray(np.concatenate([
            _ragged_pack(V, pf, nA), _ragged_pack(Vd, pf, nA),
            _ragged_pack(Vd, ps, nA), _ragged_pack(V, sg, nC),
            _ragged_pack(Vd, sg, nC)], axis=1))
        ORIG = np.full((P, 2 * nA + nC), -1, np.int64)
        ORIG[:, 0 : 2 * nA : 2] = _ragged_pack(EO, pf, nA, fill=-1)
        ORIG[:, 1 : 2 * nA : 2] = _ragged_pack(EO, ps, nA, fill=-1)
        ORIG[:, 2 * nA :] = _ragged_pack(EO, sg, nC, fill=-1)
        packed.append((idx, ORIG))
    return packed, nA, nC


_NC_CACHE = {}


def run(inputs, trace=False, **kw):
    z = np.ascontiguousarray(np.asarray(inputs["z"], dtype=np.float32))
    w1 = np.ascontiguousarray(np.asarray(inputs["W1"], dtype=np.float32))
    b1v = np.ascontiguousarray(np.asarray(inputs["b1"], dtype=np.float32))
    w2 = np.ascontiguousarray(np.asarray(inputs["W2"], dtype=np.float32))
    b2v = np.ascontiguousarray(np.asarray(inputs["b2"], dtype=np.float32))
    packed, nA, nC = pack_all(inputs["edge_label_index"])
    key = (nA, nC)
    if key not in _NC_CACHE:
        _NC_CACHE[key] = build_program(nA, nC)
    res = run_bass_kernel_spmd(
        _NC_CACHE[key],
        [{"z": z, "idx": idx, "w1": w1, "b1": b1v, "w2": w2, "b2": b2v}
         for idx, _ in packed],
        list(range(NCORES)), trace=trace, **kw)
    outs = []
    for c in range(NCORES):
        dev = res.results[c]["out"]
        orig_flat = packed[c][1].T.ravel()
        valid = orig_flat >= 0
        full = np.zeros(S_LANE * P, np.float32)
        full[orig_flat[valid]] = dev[valid]
        outs.append(full[:E_CORE])
    return np.concatenate(outs).astype(np.float32), res


def kernel(z, edge_label_index, W1, b1, W2, b2):
    out, _ = run({"z": z, "edge_label_index": edge_label_index,
                  "W1": W1, "b1": b1, "W2": W2, "b2": b2})
    return out



# revision 5
# speedup vs baseline: 1.4534x; 1.4534x over previous
"""Window-gather variant: edges sorted by src, grouped by 512-row z windows.
Src rows are rebuilt on-chip: PE one-hot matmuls (sel = is_equal(iota, srcrel)
generated on DVE) against sequentially-streamed z window tiles, costing zero
Pool-queue time. Only the dst side uses indirect DMA gathers (the ~1.45us/instr
SWDGE serialization wall). Slot counts per group are maxed across cores so one
SPMD program serves all 8. z stored bf16; bf16 PE path. Host unpermutes via
orig-id map."""

import numpy as np
import ml_dtypes

import concourse.bass as bass
import concourse.mybir as mybir
import concourse.tile as tile
from concourse import bacc
from concourse.bass import IndirectOffsetOnAxis
from concourse.bass_utils import run_bass_kernel_spmd
from concourse.masks import make_identity
from contextlib import ExitStack

N, D, H = 100000, 128, 128
E_TOTAL = 2000000
NCORES = 8
P = 128
GW = 4          # windows per group
GROUP_ROWS = GW * P   # 512 z-rows per group
NG = -(-N // GROUP_ROWS)  # 196 groups
NPAD = NG * GROUP_ROWS    # z padded to 100352 rows
E_CORE = E_TOTAL // NCORES
SLM = 16        # max slots per group (assert at pack time)

F32 = mybir.dt.float32
BF16 = mybir.dt.bfloat16
I32 = mybir.dt.int32
RELU = mybir.ActivationFunctionType.Relu
IDENT = mybir.ActivationFunctionType.Identity
EQ = mybir.AluOpType.is_equal


def build_program(counts, tot):
    nc = bacc.Bacc("TRN2", target_bir_lowering=False, debug=False,
                   enable_asserts=False, num_devices=NCORES)
    z_d = nc.dram_tensor("z", [NPAD, D], BF16, kind="ExternalInput").ap()
    idx_d = nc.dram_tensor("idx", [P, tot], I32, kind="ExternalInput").ap()
    srcrel_d = nc.dram_tensor("srcrel", [tot * P], F32, kind="ExternalInput").ap()
    w1_d = nc.dram_tensor("w1", [D, H], F32, kind="ExternalInput").ap()
    b1_d = nc.dram_tensor("b1", [H], F32, kind="ExternalInput").ap()
    w2_d = nc.dram_tensor("w2", [H, 1], F32, kind="ExternalInput").ap()
    b2_d = nc.dram_tensor("b2", [1], F32, kind="ExternalInput").ap()
    out_d = nc.dram_tensor("out", [tot * P], F32, kind="ExternalOutput").ap()

    with tile.TileContext(nc) as tc, ExitStack() as ctx:
        const = ctx.enter_context(tc.tile_pool(name="const", bufs=1))
        wpool = ctx.enter_context(tc.tile_pool(name="win", bufs=3))
        stp = ctx.enter_context(tc.tile_pool(name="srcT", bufs=2))
        selp = ctx.enter_context(tc.tile_pool(name="sel", bufs=2))
        zdp = ctx.enter_context(tc.tile_pool(name="zd", bufs=3))
        efp = ctx.enter_context(tc.tile_pool(name="ef", bufs=2))
        work = ctx.enter_context(tc.tile_pool(name="work", bufs=3))
        stage = ctx.enter_context(tc.tile_pool(name="stage", bufs=2))
        ps_om = ctx.enter_context(tc.tile_pool(name="ps_om", bufs=2, space="PSUM"))
        ps_s = ctx.enter_context(tc.tile_pool(name="ps_s", bufs=2, space="PSUM"))
        ps_t = ctx.enter_context(tc.tile_pool(name="ps_t", bufs=1, space="PSUM"))
        ps_h = ctx.enter_context(tc.tile_pool(name="ps_h", bufs=2, space="PSUM"))
        ps_o = ctx.enter_context(tc.tile_pool(name="ps_o", bufs=1, space="PSUM"))

        idx_sb = const.tile([P, tot], I32)
        nc.sync.dma_start(out=idx_sb[:], in_=idx_d[:, :])
        w1f = const.tile([P, H], F32)
        nc.sync.dma_start(out=w1f[:], in_=w1_d[:, :])
        b1_sb = const.tile([P, 1], F32)
        nc.sync.dma_start(out=b1_sb[:], in_=b1_d[:, None])
        w2f = const.tile([P, 1], F32)
        nc.sync.dma_start(out=w2f[:], in_=w2_d[:, :])
        b2_sb = const.tile([1, 1], F32)
        nc.sync.dma_start(out=b2_sb[:1], in_=b2_d[:, None])
        w1_sb = const.tile([P, H], BF16)
        nc.vector.tensor_copy(out=w1_sb[:], in_=w1f[:])
        w2_sb = const.tile([P, 1], BF16)
        nc.vector.tensor_copy(out=w2_sb[:], in_=w2f[:])
        ident = const.tile([P, P], BF16)
        make_identity(nc, ident[:])
        ones_sb = const.tile([1, P], F32)
        nc.gpsimd.memset(ones_sb[:1], 1.0)
        iota4 = const.tile([P, GW], F32)
        for j in range(GW):
            nc.gpsimd.iota(out=iota4[:, j : j + 1], pattern=[[0, 1]], base=P * j,
                           channel_multiplier=1,
                           allow_small_or_imprecise_dtypes=True)

        def gather(dst_ap, col0):
            nc.gpsimd.indirect_dma_start(
                out=dst_ap, out_offset=None, in_=z_d[:, :],
                in_offset=IndirectOffsetOnAxis(ap=idx_sb[:, col0 : col0 + 1],
                                               axis=0),
            )

        slotbase = 0
        for g in range(NG):
            cnt = counts[g]
            if cnt == 0:
                continue
            win = wpool.tile([P, GW * D], BF16, tag="win")
            for j in range(GW):
                r0 = GROUP_ROWS * g + P * j
                nc.sync.dma_start(out=win[:, j * D : (j + 1) * D],
                                  in_=z_d[r0 : r0 + P, :])
            srcT = stp.tile([1, SLM * P], F32, tag="srcT")
            nc.sync.dma_start(
                out=srcT[:1, : cnt * P],
                in_=srcrel_d[slotbase * P : (slotbase + cnt) * P][None, :])
            o_stage = stage.tile([1, SLM * P], F32, tag="ost")
            for b0 in range(0, cnt, 4):
                nb = min(4, cnt - b0)
                EB = nb * P
                som = ps_om.tile([P, 512], F32)
                nc.tensor.matmul(out=som[:, :EB], lhsT=ones_sb[:1, :],
                                 rhs=srcT[:1, b0 * P : (b0 + nb) * P],
                                 start=True, stop=True)
                sel = selp.tile([P, GW * 512], BF16, tag="sel")
                for j in range(GW):
                    nc.vector.tensor_tensor(
                        out=sel[:, j * 512 : j * 512 + EB],
                        in0=iota4[:, j : j + 1].to_broadcast([P, EB]),
                        in1=som[:, :EB], op=EQ)
                zt = zdp.tile([P, 512], BF16, tag="zd")
                for sl in range(nb):
                    gather(zt[:, sl * D : (sl + 1) * D], slotbase + b0 + sl)
                sps = ps_s.tile([P, 512], F32)
                for sl in range(nb):
                    for j in range(GW):
                        nc.tensor.matmul(
                            out=sps[:, sl * P : (sl + 1) * P],
                            lhsT=sel[:, j * 512 + sl * P : j * 512 + (sl + 1) * P],
                            rhs=win[:, j * D : (j + 1) * D],
                            start=(j == 0), stop=(j == GW - 1))
                ef = efp.tile([P, 512], BF16, tag="ef")
                nc.vector.tensor_mul(out=ef[:, :EB], in0=sps[:, :EB],
                                     in1=zt[:, :EB])
                efT_ps = ps_t.tile([P, 512], BF16)
                for c in range(nb):
                    nc.tensor.transpose(
                        out=efT_ps[:, c * P : (c + 1) * P],
                        in_=ef[:, c * D : (c + 1) * D], identity=ident[:])
                efT = work.tile([P, 512], BF16, tag="efT")
                nc.vector.tensor_copy(out=efT[:, :EB], in_=efT_ps[:, :EB])
                h_ps = ps_h.tile([P, 512], F32)
                nc.tensor.matmul(out=h_ps[:, :EB], lhsT=w1_sb[:],
                                 rhs=efT[:, :EB], start=True, stop=True)
                h_sb = work.tile([P, 512], BF16, tag="h")
                nc.scalar.activation(out=h_sb[:, :EB], in_=h_ps[:, :EB],
                                     func=RELU, bias=b1_sb[:, :1], scale=1.0)
                o_ps = ps_o.tile([1, 512], F32)
                nc.tensor.matmul(out=o_ps[:1, :EB], lhsT=w2_sb[:],
                                 rhs=h_sb[:, :EB], start=True, stop=True)
                nc.scalar.activation(
                    out=o_stage[:1, b0 * P : b0 * P + EB], in_=o_ps[:1, :EB],
                    func=IDENT, bias=b2_sb[:1, :1], scale=1.0)
            nc.sync.dma_start(
                out=out_d[slotbase * P : (slotbase + cnt) * P][None, :],
                in_=o_stage[:1, : cnt * P])
            slotbase += cnt

    nc.compile()
    return nc


def pack_all(edge_label_index, e_core=E_CORE):
    """Per-core (idx [P,tot] i32, ORIG [P,tot] i64, srcrel [tot*P] f32)
    + uniform per-group slot counts + tot."""
    src_f = np.asarray(edge_label_index[0], dtype=np.int64)
    dst_f = np.asarray(edge_label_index[1], dtype=np.int64)
    ncores = len(src_f) // e_core
    per_core = []
    counts = np.zeros(NG, np.int64)
    for c in range(ncores):
        sl = slice(c * e_core, (c + 1) * e_core)
        s, d = src_f[sl], dst_f[sl]
        order = np.argsort(s, kind="stable")
        ss, dd = s[order], d[order]
        bounds = np.searchsorted(ss, np.arange(1, NG + 1) * GROUP_ROWS)
        starts = np.concatenate([[0], bounds[:-1]])
        m = bounds - starts
        counts = np.maximum(counts, -(-m // P))
        per_core.append((ss, dd, order, starts, m))
    assert counts.max() <= SLM, counts.max()
    tot = int(counts.sum())
    packed = []
    for ss, dd, order, starts, m in per_core:
        idx = np.zeros((P, tot), np.int32)
        ORIG = np.full((P, tot), -1, np.int64)
        srcrel = np.full(tot * P, -1.0, np.float32)
        sb = 0
        for g in range(NG):
            cnt = int(counts[g])
            if cnt == 0:
                continue
            k = int(m[g])
            st = int(starts[g])
            pos = sb * P + np.arange(k)   # flat slot-major positions
            srcrel[pos] = (ss[st : st + k] - GROUP_ROWS * g).astype(np.float32)
            lanes = np.arange(k) % P
            slots = sb + np.arange(k) // P
            idx[lanes, slots] = dd[st : st + k]
            ORIG[lanes, slots] = order[st : st + k]
            sb += cnt
        packed.append((idx, ORIG, srcrel))
    return packed, counts, tot


_NC_CACHE = {}


def run(inputs, trace=False, **kw):
    z = np.asarray(inputs["z"], dtype=np.float32)
    zp = np.zeros((NPAD, D), np.float32)
    zp[:N] = z
    zb = np.ascontiguousarray(zp.astype(ml_dtypes.bfloat16))
    w1 = np.ascontiguousarray(np.asarray(inputs["W1"], dtype=np.float32))
    b1v = np.ascontiguousarray(np.asarray(inputs["b1"], dtype=np.float32))
    w2 = np.ascontiguousarray(np.asarray(inputs["W2"], dtype=np.float32))
    b2v = np.ascontiguousarray(np.asarray(inputs["b2"], dtype=np.float32))
    packed, counts, tot = pack_all(inputs["edge_label_index"])
    key = (tuple(counts), tot)
    if key not in _NC_CACHE:
        _NC_CACHE[key] = build_program(counts, tot)
    res = run_bass_kernel_spmd(
        _NC_CACHE[key],
        [{"z": zb, "idx": idx, "srcrel": srcrel, "w1": w1, "b1": b1v,
          "w2": w2, "b2": b2v}
         for idx, _, srcrel in packed],
        list(range(NCORES)), trace=trace, **kw)
    outs = []
    for c in range(NCORES):
        dev = res.results[c]["out"]
        orig_flat = packed[c][1].T.ravel()
        valid = orig_flat >= 0
        full = np.zeros(E_CORE, np.float32)
        full[orig_flat[valid]] = dev[valid]
        outs.append(full)
    return np.concatenate(outs).astype(np.float32), res


def kernel(z, edge_label_index, W1, b1, W2, b2):
    out, _ = run({"z": z, "edge_label_index": edge_label_index,
                  "W1": W1, "b1": b1, "W2": W2, "b2": b2})
    return out


# revision 6
# speedup vs baseline: 1.5766x; 1.0848x over previous
"""Window-gather variant: edges sorted by src, grouped by 512-row z windows.
Src rows are rebuilt on-chip: PE one-hot matmuls (sel = is_equal(iota, srcrel)
generated on DVE) against sequentially-streamed z window tiles, costing zero
Pool-queue time. Only the dst side uses indirect DMA gathers (the ~1.45us/instr
SWDGE serialization wall). Slot counts per group are maxed across cores so one
SPMD program serves all 8. z stored bf16; bf16 PE path. Host unpermutes via
orig-id map."""

import numpy as np
import ml_dtypes

import concourse.bass as bass
import concourse.mybir as mybir
import concourse.tile as tile
from concourse import bacc
from concourse.bass import IndirectOffsetOnAxis
from concourse.bass_utils import run_bass_kernel_spmd
from concourse.masks import make_identity
from contextlib import ExitStack

N, D, H = 100000, 128, 128
E_TOTAL = 2000000
NCORES = 8
P = 128
GW = 4          # windows per group
GROUP_ROWS = GW * P   # 512 z-rows per group
NG = -(-N // GROUP_ROWS)  # 196 groups
NPAD = NG * GROUP_ROWS    # z padded to 100352 rows
E_CORE = E_TOTAL // NCORES
SLM = 16        # max slots per group (assert at pack time)

F32 = mybir.dt.float32
BF16 = mybir.dt.bfloat16
I32 = mybir.dt.int32
RELU = mybir.ActivationFunctionType.Relu
IDENT = mybir.ActivationFunctionType.Identity
EQ = mybir.AluOpType.is_equal


def build_program(counts, tot):
    nc = bacc.Bacc("TRN2", target_bir_lowering=False, debug=False,
                   enable_asserts=False, num_devices=NCORES)
    z_d = nc.dram_tensor("z", [NPAD, D], BF16, kind="ExternalInput").ap()
    idx_d = nc.dram_tensor("idx", [P, tot], I32, kind="ExternalInput").ap()
    srcrel_d = nc.dram_tensor("srcrel", [tot * P], F32, kind="ExternalInput").ap()
    w1_d = nc.dram_tensor("w1", [D, H], F32, kind="ExternalInput").ap()
    b1_d = nc.dram_tensor("b1", [H], F32, kind="ExternalInput").ap()
    w2_d = nc.dram_tensor("w2", [H, 1], F32, kind="ExternalInput").ap()
    b2_d = nc.dram_tensor("b2", [1], F32, kind="ExternalInput").ap()
    out_d = nc.dram_tensor("out", [tot * P], F32, kind="ExternalOutput").ap()

    with tile.TileContext(nc) as tc, ExitStack() as ctx:
        const = ctx.enter_context(tc.tile_pool(name="const", bufs=1))
        wpool = ctx.enter_context(tc.tile_pool(name="win", bufs=3))
        stp = ctx.enter_context(tc.tile_pool(name="srcT", bufs=2))
        selp = ctx.enter_context(tc.tile_pool(name="sel", bufs=2))
        zdp = ctx.enter_context(tc.tile_pool(name="zd", bufs=3))
        efp = ctx.enter_context(tc.tile_pool(name="ef", bufs=2))
        work = ctx.enter_context(tc.tile_pool(name="work", bufs=3))
        stage = ctx.enter_context(tc.tile_pool(name="stage", bufs=2))
        ps_om = ctx.enter_context(tc.tile_pool(name="ps_om", bufs=2, space="PSUM"))
        ps_s = ctx.enter_context(tc.tile_pool(name="ps_s", bufs=2, space="PSUM"))
        ps_t = ctx.enter_context(tc.tile_pool(name="ps_t", bufs=1, space="PSUM"))
        ps_h = ctx.enter_context(tc.tile_pool(name="ps_h", bufs=2, space="PSUM"))
        ps_o = ctx.enter_context(tc.tile_pool(name="ps_o", bufs=1, space="PSUM"))

        idx_sb = const.tile([P, tot], I32)
        nc.sync.dma_start(out=idx_sb[:], in_=idx_d[:, :])
        w1f = const.tile([P, H], F32)
        nc.sync.dma_start(out=w1f[:], in_=w1_d[:, :])
        b1_sb = const.tile([P, 1], F32)
        nc.sync.dma_start(out=b1_sb[:], in_=b1_d[:, None])
        w2f = const.tile([P, 1], F32)
        nc.sync.dma_start(out=w2f[:], in_=w2_d[:, :])
        b2_sb = const.tile([1, 1], F32)
        nc.sync.dma_start(out=b2_sb[:1], in_=b2_d[:, None])
        w1_sb = const.tile([P, H], BF16)
        nc.vector.tensor_copy(out=w1_sb[:], in_=w1f[:])
        w2_sb = const.tile([P, 1], BF16)
        nc.vector.tensor_copy(out=w2_sb[:], in_=w2f[:])
        ident = const.tile([P, P], BF16)
        make_identity(nc, ident[:])
        ones_sb = const.tile([1, P], F32)
        nc.gpsimd.memset(ones_sb[:1], 1.0)
        iota4 = const.tile([P, GW], F32)
        for j in range(GW):
            nc.gpsimd.iota(out=iota4[:, j : j + 1], pattern=[[0, 1]], base=P * j,
                           channel_multiplier=1,
                           allow_small_or_imprecise_dtypes=True)

        def gather(dst_ap, col0):
            nc.gpsimd.indirect_dma_start(
                out=dst_ap, out_offset=None, in_=z_d[:, :],
                in_offset=IndirectOffsetOnAxis(ap=idx_sb[:, col0 : col0 + 1],
                                               axis=0),
            )

        slotbase = 0
        for g in range(NG):
            cnt = counts[g]
            if cnt == 0:
                continue
            win = wpool.tile([P, GW * D], BF16, tag="win")
            for j in range(GW):
                r0 = GROUP_ROWS * g + P * j
                nc.sync.dma_start(out=win[:, j * D : (j + 1) * D],
                                  in_=z_d[r0 : r0 + P, :])
            srcT = stp.tile([1, SLM * P], F32, tag="srcT")
            nc.sync.dma_start(
                out=srcT[:1, : cnt * P],
                in_=srcrel_d[slotbase * P : (slotbase + cnt) * P][None, :])
            o_stage = stage.tile([1, SLM * P], F32, tag="ost")
            for b0 in range(0, cnt, 4):
                nb = min(4, cnt - b0)
                EB = nb * P
                som = ps_om.tile([P, 512], F32)
                nc.tensor.matmul(out=som[:, :EB], lhsT=ones_sb[:1, :],
                                 rhs=srcT[:1, b0 * P : (b0 + nb) * P],
                                 start=True, stop=True)
                sel = selp.tile([P, GW * 512], BF16, tag="sel")
                for j in range(GW):
                    nc.vector.tensor_tensor(
                        out=sel[:, j * 512 : j * 512 + EB],
                        in0=iota4[:, j : j + 1].to_broadcast([P, EB]),
                        in1=som[:, :EB], op=EQ)
                zt = zdp.tile([P, 512], BF16, tag="zd")
                for sl in range(nb):
                    gather(zt[:, sl * D : (sl + 1) * D], slotbase + b0 + sl)
                ztT_ps = ps_t.tile([P, 512], BF16)
                for c in range(nb):
                    nc.tensor.transpose(
                        out=ztT_ps[:, c * P : (c + 1) * P],
                        in_=zt[:, c * D : (c + 1) * D], identity=ident[:])
                ztT = work.tile([P, 512], BF16, tag="ztT")
                nc.vector.tensor_copy(out=ztT[:, :EB], in_=ztT_ps[:, :EB])
                spsT = ps_s.tile([P, 512], F32)
                for j in range(GW):
                    nc.tensor.matmul(
                        out=spsT[:, :EB], lhsT=win[:, j * D : (j + 1) * D],
                        rhs=sel[:, j * 512 : j * 512 + EB],
                        start=(j == 0), stop=(j == GW - 1))
                efT = efp.tile([P, 512], BF16, tag="ef")
                nc.vector.tensor_mul(out=efT[:, :EB], in0=spsT[:, :EB],
                                     in1=ztT[:, :EB])
                h_ps = ps_h.tile([P, 512], F32)
                nc.tensor.matmul(out=h_ps[:, :EB], lhsT=w1_sb[:],
                                 rhs=efT[:, :EB], start=True, stop=True)
                h_sb = work.tile([P, 512], BF16, tag="h")
                nc.scalar.activation(out=h_sb[:, :EB], in_=h_ps[:, :EB],
                                     func=RELU, bias=b1_sb[:, :1], scale=1.0)
                o_ps = ps_o.tile([1, 512], F32)
                nc.tensor.matmul(out=o_ps[:1, :EB], lhsT=w2_sb[:],
                                 rhs=h_sb[:, :EB], start=True, stop=True)
                nc.scalar.activation(
                    out=o_stage[:1, b0 * P : b0 * P + EB], in_=o_ps[:1, :EB],
                    func=IDENT, bias=b2_sb[:1, :1], scale=1.0)
            nc.sync.dma_start(
                out=out_d[slotbase * P : (slotbase + cnt) * P][None, :],
                in_=o_stage[:1, : cnt * P])
            slotbase += cnt

    nc.compile()
    return nc


def pack_all(edge_label_index, e_core=E_CORE):
    """Per-core (idx [P,tot] i32, ORIG [P,tot] i64, srcrel [tot*P] f32)
    + uniform per-group slot counts + tot."""
    src_f = np.asarray(edge_label_index[0], dtype=np.int64)
    dst_f = np.asarray(edge_label_index[1], dtype=np.int64)
    ncores = len(src_f) // e_core
    per_core = []
    counts = np.zeros(NG, np.int64)
    for c in range(ncores):
        sl = slice(c * e_core, (c + 1) * e_core)
        s, d = src_f[sl], dst_f[sl]
        order = np.argsort(s, kind="stable")
        ss, dd = s[order], d[order]
        bounds = np.searchsorted(ss, np.arange(1, NG + 1) * GROUP_ROWS)
        starts = np.concatenate([[0], bounds[:-1]])
        m = bounds - starts
        counts = np.maximum(counts, -(-m // P))
        per_core.append((ss, dd, order, starts, m))
    assert counts.max() <= SLM, counts.max()
    tot = int(counts.sum())
    packed = []
    for ss, dd, order, starts, m in per_core:
        idx = np.zeros((P, tot), np.int32)
        ORIG = np.full((P, tot), -1, np.int64)
        srcrel = np.full(tot * P, -1.0, np.float32)
        sb = 0
        for g in range(NG):
            cnt = int(counts[g])
            if cnt == 0:
                continue
            k = int(m[g])
            st = int(starts[g])
            pos = sb * P + np.arange(k)   # flat slot-major positions
            srcrel[pos] = (ss[st : st + k] - GROUP_ROWS * g).astype(np.float32)
            lanes = np.arange(k) % P
            slots = sb + np.arange(k) // P
            idx[lanes, slots] = dd[st : st + k]
            ORIG[lanes, slots] = order[st : st + k]
            sb += cnt
        packed.append((idx, ORIG, srcrel))
    return packed, counts, tot


_NC_CACHE = {}


def run(inputs, trace=False, **kw):
    z = np.asarray(inputs["z"], dtype=np.float32)
    zp = np.zeros((NPAD, D), np.float32)
    zp[:N] = z
    zb = np.ascontiguousarray(zp.astype(ml_dtypes.bfloat16))
    w1 = np.ascontiguousarray(np.asarray(inputs["W1"], dtype=np.float32))
    b1v = np.ascontiguousarray(np.asarray(inputs["b1"], dtype=np.float32))
    w2 = np.ascontiguousarray(np.asarray(inputs["W2"], dtype=np.float32))
    b2v = np.ascontiguousarray(np.asarray(inputs["b2"], dtype=np.float32))
    packed, counts, tot = pack_all(inputs["edge_label_index"])
    key = (tuple(counts), tot)
    if key not in _NC_CACHE:
        _NC_CACHE[key] = build_program(counts, tot)
    res = run_bass_kernel_spmd(
        _NC_CACHE[key],
        [{"z": zb, "idx": idx, "srcrel": srcrel, "w1": w1, "b1": b1v,
          "w2": w2, "b2": b2v}
         for idx, _, srcrel in packed],
        list(range(NCORES)), trace=trace, **kw)
    outs = []
    for c in range(NCORES):
        dev = res.results[c]["out"]
        orig_flat = packed[c][1].T.ravel()
        valid = orig_flat >= 0
        full = np.zeros(E_CORE, np.float32)
        full[orig_flat[valid]] = dev[valid]
        outs.append(full)
    return np.concatenate(outs).astype(np.float32), res


def kernel(z, edge_label_index, W1, b1, W2, b2):
    out, _ = run({"z": z, "edge_label_index": edge_label_index,
                  "W1": W1, "b1": b1, "W2": W2, "b2": b2})
    return out
